# revision 1
# baseline (speedup 1.0000x reference)
"""CNN-MRF loss (retrieval kNN) on 8 Trainium2 NeuronCores.

Reference: cosine-similarity argmax between all 96x96 content patches and
96x96 style patches (3x3xC=128 patches, d=1152), gather matched style
patches, fold (overlap-add), MSE against content features.

Sharding: content-patch axis N split 8 ways (12 grid rows / core), style
replicated.  Two-pass retrieval per core:
  coarse: bf16 similarity S (128 content x 384 style tiles) = sum of 9
     shifted matmuls (contraction = channels on partitions) accumulated
     in PSUM, scaled by replicated 1/||s||, stored bf16; DVE max8 +
     find_index8 give the top-8 candidate style patches per content patch.
     bf16 quantization (~3e-4) is far below the top-8 margin (~5e-2), so
     the true argmax is always among the 8.
  rescore: indirect-DMA gather of the 8 candidate patch rows (fp32),
     exact fp32 dot x 1/||s|| on DVE, one-hot select of the winner.
  then: indirect-DMA gather of the matched (un-normalized) style patch
     rows, PE transposes to channel-major, DVE fold accumulation into a
     14-row output strip.
Host: sums the 8 overlapping strips, divides by fold counts, MSE.
"""
import sys
import numpy as np

for _p in ("/opt/trn_rl_repo",):
    if _p not in sys.path:
        sys.path.insert(0, _p)

import concourse.bass as bass
import concourse.bacc as bacc
import concourse.mybir as mybir
from concourse.bass import IndirectOffsetOnAxis
from concourse.bass_utils import run_bass_kernel_spmd
from concourse.tile import TileContext
from concourse.masks import make_identity

F32 = mybir.dt.float32
BF16 = mybir.dt.bfloat16
U32 = mybir.dt.uint32

C = 128          # channels
H = W = 96       # feature-map spatial dims
PW = 3           # patch size
HP = H + 2       # padded spatial
N = H * W        # content patches total (9216)
M = N            # style patches (9216)
D = C * PW * PW  # patch vector length (1152)
NCORES = 8
RPC = H // NCORES       # content grid rows per core (12)
NSH = RPC * W           # content patches per core (1152)
NT = NSH // 128         # n-tiles of 128 per core (9)
MROWS = 4               # style grid rows per m-tile
MW = MROWS * W          # m-tile width (384)
MT = M // MW            # m-tiles (24)
import os
TOPK = 8
RL = int(os.environ.get("RL", "0"))  # 0=coarse, 1=full rescore, 2=dots-only
RESCORE = RL >= 1


def ts(i, size):
    return slice(i * size, (i + 1) * size)


def build_program():
    nc = bacc.Bacc()

    cpad_bf = nc.declare_dram_parameter(
        "cpad_bf", [C, RPC + 2, HP], BF16, isOutput=False
    )
    spad_bf = nc.declare_dram_parameter("spad_bf", [C, HP, HP], BF16, isOutput=False)
    sprows = nc.declare_dram_parameter("sprows", [M, D], F32, isOutput=False)
    cprows = nc.declare_dram_parameter("cprows", [NSH, D], F32, isOutput=False)
    invn_row = nc.declare_dram_parameter("invn_row", [1, M], F32, isOutput=False)
    idx_out = nc.declare_dram_parameter("idx_out", [NT, 128, 1], U32, isOutput=True)
    racc_out = nc.declare_dram_parameter(
        "racc_out", [C, RPC + 2, W], F32, isOutput=True
    )

    with TileContext(nc) as tc:
        with (
            tc.tile_pool(name="const", bufs=1) as constp,
            tc.tile_pool(name="big", bufs=1) as bigp,
            tc.tile_pool(name="work", bufs=2) as workp,
            tc.tile_pool(name="psS", bufs=4, space="PSUM") as psS,
            tc.tile_pool(name="psT", bufs=2, space="PSUM") as psT,
            tc.tile_pool(name="psN", bufs=2, space="PSUM") as psN,
        ):
            # ---- constants / loads ----
            ones_row = constp.tile([1, 128], F32)     # for partition broadcast
            nc.vector.memset(ones_row[:], 1.0)
            ident = constp.tile([128, 128], F32)
            make_identity(nc, ident[:])

            spad_t = bigp.tile([C, HP, HP], BF16)
            nc.sync.dma_start(out=spad_t[:], in_=spad_bf[:])
            cpad_t = bigp.tile([C, RPC + 2, HP], BF16)
            nc.sync.dma_start(out=cpad_t[:], in_=cpad_bf[:])

            # ---- style inverse norms, partition-broadcast: invb (128, M) ----
            invb = bigp.tile([C, M], F32)
            for t in range(MT):
                invn_t = workp.tile([1, MW], F32, tag="invn")
                nc.sync.dma_start(out=invn_t[:], in_=invn_row[0:1, ts(t, MW)])
                psum_b = psN.tile([128, MW], F32, tag="psb")
                nc.tensor.matmul(
                    out=psum_b[:],
                    lhsT=ones_row[:],
                    rhs=invn_t[:],
                    start=True,
                    stop=True,
                )
                nc.vector.tensor_copy(invb[:, ts(t, MW)], psum_b[:])

            # ---- contiguous shifted content views (bf16 weights) ----
            cshift = bigp.tile([C, 9, NSH], BF16)
            for k in range(9):
                ki, kj = k // 3, k % 3
                nc.vector.tensor_copy(
                    cshift[:, k], cpad_t[:, ki : ki + RPC, kj : kj + W]
                )

            # ---- coarse similarity + top-8 + rescore + gather + fold ----
            racc = bigp.tile([C, RPC + 2, HP], F32)
            nc.gpsimd.memset(racc[:], 0.0)

            MTILES = [(5 * i, 5) for i in range(19)] + [(95, 1)]
            for j in range(NT):
                S_sb = bigp.tile([C, M], BF16, tag="S_sb", bufs=2)
                cprows_j = workp.tile([128, D], F32, tag="cpr")
                nc.sync.dma_start(out=cprows_j[:], in_=cprows[ts(j, 128), :])

                for g in range(0, len(MTILES), 4):
                    grp = []
                    for (mrow, nr) in MTILES[g : g + 4]:
                        pt = psS.tile([128, 480], F32, tag="psS", name=f"ps_{j}_{g}_{mrow}")
                        grp.append((pt, mrow, nr))
                    for k in range(9):
                        ki, kj = k // 3, k % 3
                        lhsT = cshift[:, k, ts(j, 128)]
                        for (pt, mrow, nr) in grp:
                            nc.tensor.matmul(
                                out=pt[:, : nr * W],
                                lhsT=lhsT,
                                rhs=spad_t[
                                    :, mrow + ki : mrow + ki + nr, kj : kj + W
                                ],
                                start=(k == 0),
                                stop=(k == 8),
                            )
                    for (pt, mrow, nr) in grp:
                        nc.vector.tensor_mul(
                            S_sb[:, mrow * W : (mrow + nr) * W],
                            pt[:, : nr * W],
                            invb[:, mrow * W : (mrow + nr) * W],
                        )
                max8 = workp.tile([128, 8], BF16, tag="max8")
                nc.vector.max(max8[:], S_sb[:])
                idx8 = workp.tile([128, 8], U32, tag="idx8")
                nc.vector.max_index(idx8[:], max8[:], S_sb[:])

                if RESCORE:
                    # ---- exact fp32 rescore of the 8 candidates ----
                    use_rescore = RESCORE
                    dots8 = workp.tile([128, 8], F32, tag="dots8")
                    nrm28 = workp.tile([128, 8], F32, tag="nrm28")
                    for cc in range(TOPK):
                        idxcc = workp.tile([128, 1], U32, tag="idxcc")
                        nc.vector.tensor_copy(idxcc[:], idx8[:, cc : cc + 1])
                        gath = workp.tile([128, D], F32, tag="gath")
                        nc.gpsimd.indirect_dma_start(
                            out=gath[:],
                            out_offset=None,
                            in_=sprows[:],
                            in_offset=IndirectOffsetOnAxis(
                                ap=idxcc[:, 0:1], axis=0
                            ),
                        )
                        scr = workp.tile([128, D], F32, tag="scr")
                        scr2 = workp.tile([128, D], F32, tag="scr2")
                        nc.vector.tensor_tensor_reduce(
                            out=scr[:],
                            in0=gath[:],
                            in1=cprows_j[:],
                            scale=1.0,
                            scalar=0.0,
                            op0=mybir.AluOpType.mult,
                            op1=mybir.AluOpType.add,
                            accum_out=dots8[:, cc : cc + 1],
                        )
                        nc.vector.tensor_tensor_reduce(
                            out=scr2[:],
                            in0=gath[:],
                            in1=gath[:],
                            scale=1.0,
                            scalar=0.0,
                            op0=mybir.AluOpType.mult,
                            op1=mybir.AluOpType.add,
                            accum_out=nrm28[:, cc : cc + 1],
                        )
                    sq8 = workp.tile([128, 8], F32, tag="sq8")
                    nc.scalar.activation(
                        sq8[:], nrm28[:], mybir.ActivationFunctionType.Sqrt
                    )
                    if RL == 2:
                        bestu = workp.tile([128, 1], U32, tag="bestu")
                        nc.vector.tensor_copy(bestu[:], idx8[:, 0:1])
                        nc.sync.dma_start(out=idx_out[j], in_=bestu[:])
                    if RL == 1:
                        rec8 = workp.tile([128, 8], F32, tag="rec8")
                        nc.vector.reciprocal(rec8[:], sq8[:])
                        s8 = workp.tile([128, 8], F32, tag="s8")
                        nc.vector.tensor_mul(s8[:], dots8[:], rec8[:])
                        top8 = workp.tile([128, 8], F32, tag="top8")
                        nc.vector.max(top8[:], s8[:])
                        onehot = workp.tile([128, 8], F32, tag="onehot")
                        nc.vector.tensor_tensor(
                            out=onehot[:],
                            in0=s8[:],
                            in1=top8[:, 0:1].to_broadcast((128, 8)),
                            op=mybir.AluOpType.is_equal,
                        )
                        idx8f = workp.tile([128, 8], F32, tag="idx8f")
                        nc.vector.tensor_copy(idx8f[:], idx8[:])
                        selscr = workp.tile([128, 8], F32, tag="selscr")
                        bestf = workp.tile([128, 1], F32, tag="bestf")
                        nc.vector.tensor_tensor_reduce(
                            out=selscr[:],
                            in0=onehot[:],
                            in1=idx8f[:],
                            scale=1.0,
                            scalar=-1.0,
                            op0=mybir.AluOpType.mult,
                            op1=mybir.AluOpType.max,
                            accum_out=bestf[:],
                        )
                        bestu = workp.tile([128, 1], U32, tag="bestu")
                        nc.vector.tensor_copy(bestu[:], bestf[:])
                        nc.sync.dma_start(out=idx_out[j], in_=bestu[:])
                else:
                    bestu = workp.tile([128, 1], U32, tag="bestu")
                    nc.vector.tensor_copy(bestu[:], idx8[:, 0:1])
                    nc.sync.dma_start(out=idx_out[j], in_=bestu[:])

                # gather matched style patch rows (n-major); the indirect
                # DMA needs a flat 2D dest (3D dest tiles fetch garbage)
                matched = workp.tile([128, D], F32, tag="matched")
                nc.gpsimd.indirect_dma_start(
                    out=matched[:],
                    out_offset=None,
                    in_=sprows[:],
                    in_offset=IndirectOffsetOnAxis(ap=bestu[:, 0:1], axis=0),
                )
                matched3 = matched[:].rearrange("p (a b) -> p a b", b=9)

                # transpose to channel-major and fold-accumulate
                n0 = j * 128
                r0, c0 = n0 // W, n0 % W
                seg1 = (r0, c0, W - c0, 0)
                seg2 = (r0 + 1, 0, 128 - (W - c0), W - c0)
                for k in range(9):
                    ki, kj = k // 3, k % 3
                    psum_T = psT.tile([128, 128], F32, tag="psT")
                    nc.tensor.transpose(psum_T[:], matched3[:, :, k], ident[:])
                    for (r, c, ln, off) in (seg1, seg2):
                        nc.vector.tensor_add(
                            racc[:, r + ki, c + kj : c + kj + ln],
                            racc[:, r + ki, c + kj : c + kj + ln],
                            psum_T[:, off : off + ln],
                        )

            nc.sync.dma_start(out=racc_out[:], in_=racc[:, :, 1 : 1 + W])

    if not nc.is_finalized():
        nc.finalize()
    return nc


_PROGRAM = None


def _get_program():
    global _PROGRAM
    if _PROGRAM is None:
        _PROGRAM = build_program()
    return _PROGRAM


def _patch_rows(x):
    """(C, R, Cc) padded map -> ((R-2)*(Cc-2), C*9) patch rows, (c,ki,kj)."""
    w = np.lib.stride_tricks.sliding_window_view(x, (PW, PW), axis=(1, 2))
    return np.ascontiguousarray(
        w.transpose(1, 2, 0, 3, 4).reshape((x.shape[1] - 2) * (x.shape[2] - 2), -1)
    )


def _host_prep(content_feats, style_feats):
    """Build per-core input maps."""
    bf = mybir.dt.np(BF16)
    cf = np.ascontiguousarray(np.asarray(content_feats, dtype=np.float32)[0])
    sf = np.ascontiguousarray(np.asarray(style_feats, dtype=np.float32)[0])
    cpad = np.pad(cf, ((0, 0), (1, 1), (1, 1)))
    spad = np.pad(sf, ((0, 0), (1, 1), (1, 1)))
    sprows = _patch_rows(spad)
    spad_b = spad.astype(bf)
    invn = (
        1.0
        / np.maximum(np.linalg.norm(sprows, axis=1), np.float32(1e-12))
    ).astype(np.float32)
    in_maps = []
    for i in range(NCORES):
        cslab = np.ascontiguousarray(cpad[:, i * RPC : i * RPC + RPC + 2, :])
        in_maps.append(
            {
                "cpad_bf": cslab.astype(bf),
                "spad_bf": spad_b,
                "sprows": sprows,
                "cprows": _patch_rows(cslab),
                "invn_row": np.ascontiguousarray(invn.reshape(1, M)),
            }
        )
    return cf, in_maps


_DIVISOR = None


def _fold_divisor():
    global _DIVISOR
    if _DIVISOR is None:
        cnt = np.full(H, 3, dtype=np.float32)
        cnt[0] = cnt[-1] = 2
        _DIVISOR = np.outer(cnt, cnt).astype(np.float32) + np.float32(1e-8)
    return _DIVISOR


def _host_combine(cf, results):
    acc = np.zeros((C, H + 2, W), dtype=np.float32)
    for i in range(NCORES):
        acc[:, i * RPC : i * RPC + RPC + 2, :] += results[i]["racc_out"]
    recon = acc[:, 1 : 1 + H, :] / _fold_divisor()[None, :, :]
    diff = cf - recon
    return np.float32(np.mean(np.square(diff), dtype=np.float64))


def run(content_feats, style_feats, trace=False):
    nc = _get_program()
    cf, in_maps = _host_prep(content_feats, style_feats)
    res = run_bass_kernel_spmd(
        nc, in_maps, core_ids=list(range(NCORES)), trace=trace
    )
    mse = _host_combine(cf, res.results)
    return mse, res


def kernel(content_feats, style_feats):
    mse, _ = run(content_feats, style_feats)
    return np.array(mse, dtype=np.float32)



# revision 3
# speedup vs baseline: 1.7673x; 1.7673x over previous
"""CNN-MRF loss (retrieval kNN) on 8 Trainium2 NeuronCores.

Reference: cosine-similarity argmax between all 96x96 content patches and
96x96 style patches (3x3xC=128 patches, d=1152), gather matched style
patches, fold (overlap-add), MSE against content features.

Sharding: content-patch axis N split 8 ways (12 grid rows / core), style
replicated.  Per core:
  coarse: fp8(e4m3) DoubleRow matmuls on host-prenormalized style patch
     rows (scaled x1024) against fp8 content patch rows -- PSUM directly
     holds cosine scores (content norm is argmax-invariant).  Contraction
     1152 = 5 plane-pairs of 2x128 (10th plane zeros).  18 m-groups of
     512 columns, PSUM double-bank tiles, drained to bf16 S rows by the
     Scalar engine.  fp8 coarse top-1 vs exact argmax: ~6% of patches
     pick a near-tied neighbor; MSE rel err ~6e-5 (validated on host).
  argmax: DVE 3D reduce-max over [128,18,512] -> group maxes, max8 over
     18 -> global max, one full-row max_index8 -> index.
  then: indirect-DMA gather of the matched (un-normalized fp32) style
     patch rows, PE transposes to channel-major, DVE fold accumulation
     into a 14-row output strip.
Host: sums the 8 overlapping strips, divides by fold counts, MSE.
"""
import sys
import numpy as np

for _p in ("/opt/trn_rl_repo",):
    if _p not in sys.path:
        sys.path.insert(0, _p)

import concourse.bass as bass
import concourse.bacc as bacc
import concourse.mybir as mybir
from concourse.bass import IndirectOffsetOnAxis
from concourse.bass_utils import run_bass_kernel_spmd
from concourse.tile import TileContext
from concourse.masks import make_identity

F32 = mybir.dt.float32
BF16 = mybir.dt.bfloat16
F8 = mybir.dt.float8e4
U32 = mybir.dt.uint32

C = 128          # channels
H = W = 96       # feature-map spatial dims
PW = 3           # patch size
HP = H + 2       # padded spatial
N = H * W        # content patches total (9216)
M = N            # style patches (9216)
D = C * PW * PW  # patch vector length (1152)
NCORES = 8
RPC = H // NCORES       # content grid rows per core (12)
NSH = RPC * W           # content patches per core (1152)
NT = NSH // 128         # n-tiles of 128 per core (9)
GW = 512                # m-group width (one PSUM bank of fp32)
NG = M // GW            # m-groups (18)
SG = 3                  # supergroups of 6 groups (3 double-bank tiles)
SCALE = 1024.0          # fp8 quantization scale for normalized style rows


def ts(i, size):
    return slice(i * size, (i + 1) * size)


def build_program():
    nc = bacc.Bacc()

    snorm8 = nc.declare_dram_parameter("snorm8", [C, 9, M], F8, isOutput=False)
    cp8 = nc.declare_dram_parameter("cp8", [C, 10, NSH], F8, isOutput=False)
    sprows = nc.declare_dram_parameter("sprows", [M, D], F32, isOutput=False)
    idx_out = nc.declare_dram_parameter("idx_out", [NT, 128, 1], U32, isOutput=True)
    racc_out = nc.declare_dram_parameter(
        "racc_out", [C, RPC + 2, W], F32, isOutput=True
    )

    DR = mybir.MatmulPerfMode.DoubleRow
    Copy = mybir.ActivationFunctionType.Copy

    with TileContext(nc) as tc:
        with (
            tc.tile_pool(name="const", bufs=1) as constp,
            tc.tile_pool(name="big", bufs=1) as bigp,
            tc.tile_pool(name="work", bufs=2) as workp,
            tc.tile_pool(name="psD", bufs=3, space="PSUM") as psD,
            tc.tile_pool(name="psT", bufs=2, space="PSUM") as psT,
        ):
            ident = constp.tile([128, 128], F32)
            make_identity(nc, ident[:])

            # style rows: [c, plane, m], plane 9 zeroed (pair-pad for DoubleRow)
            snorm_t = bigp.tile([C, 10, M], F8)
            nc.gpsimd.memset(snorm_t[:, 9], 0.0)
            for g in range(NG):
                nc.sync.dma_start(
                    out=snorm_t[:, 0:9, ts(g, GW)], in_=snorm8[:, :, ts(g, GW)]
                )
            cp_t = bigp.tile([C, 10, NSH], F8)
            nc.sync.dma_start(out=cp_t[:], in_=cp8[:])

            racc = bigp.tile([C, RPC + 2, HP], F32)
            nc.gpsimd.memset(racc[:], 0.0)

            for j in range(NT):
                # coarse scores, bf16, [128 content x 9216 style]
                S_sb = bigp.tile([C, NG * GW], BF16, tag="S_sb", bufs=2)
                for sg in range(SG):
                    pds = [
                        psD.tile(
                            [128, 2 * GW], F32, tag="psD", name=f"pd_{j}_{sg}_{t2}"
                        )
                        for t2 in range(3)
                    ]
                    for kp in range(5):
                        lhsT = cp_t[:, 2 * kp : 2 * kp + 2, ts(j, 128)]
                        for t2 in range(3):
                            for h in range(2):
                                g = sg * 6 + t2 * 2 + h
                                nc.tensor.matmul(
                                    out=pds[t2][:, ts(h, GW)],
                                    lhsT=lhsT,
                                    rhs=snorm_t[:, 2 * kp : 2 * kp + 2, ts(g, GW)],
                                    start=(kp == 0),
                                    stop=(kp == 4),
                                    perf_mode=DR,
                                )
                    for t2 in range(3):
                        g0 = sg * 6 + t2 * 2
                        nc.scalar.activation(
                            S_sb[:, g0 * GW : (g0 + 2) * GW], pds[t2][:], Copy
                        )

                # hierarchical argmax over 9216 bf16 scores
                gmax = workp.tile([128, NG], BF16, tag="gmax")
                nc.vector.tensor_reduce(
                    gmax[:],
                    S_sb[:].rearrange("p (a b) -> p a b", b=GW),
                    axis=mybir.AxisListType.X,
                    op=mybir.AluOpType.max,
                )
                gmax8 = workp.tile([128, 8], BF16, tag="gmax8")
                nc.vector.max(gmax8[:], gmax[:])
                idx8 = workp.tile([128, 8], U32, tag="idx8")
                nc.vector.max_index(idx8[:], gmax8[:], S_sb[:])
                bestu = workp.tile([128, 1], U32, tag="bestu")
                nc.vector.tensor_copy(bestu[:], idx8[:, 0:1])
                nc.sync.dma_start(out=idx_out[j], in_=bestu[:])

                # gather matched (un-normalized) style patch rows (n-major)
                matched = workp.tile([128, D], F32, tag="matched")
                nc.gpsimd.indirect_dma_start(
                    out=matched[:],
                    out_offset=None,
                    in_=sprows[:],
                    in_offset=IndirectOffsetOnAxis(ap=bestu[:, 0:1], axis=0),
                )
                matched3 = matched[:].rearrange("p (a b) -> p a b", b=9)

                # transpose to channel-major and fold-accumulate
                n0 = j * 128
                r0, c0 = n0 // W, n0 % W
                seg1 = (r0, c0, W - c0, 0)
                seg2 = (r0 + 1, 0, 128 - (W - c0), W - c0)
                for k in range(9):
                    ki, kj = k // 3, k % 3
                    psum_T = psT.tile([128, 128], F32, tag="psT")
                    nc.tensor.transpose(psum_T[:], matched3[:, :, k], ident[:])
                    for (r, c, ln, off) in (seg1, seg2):
                        nc.vector.tensor_add(
                            racc[:, r + ki, c + kj : c + kj + ln],
                            racc[:, r + ki, c + kj : c + kj + ln],
                            psum_T[:, off : off + ln],
                        )

            nc.sync.dma_start(out=racc_out[:], in_=racc[:, :, 1 : 1 + W])

    if not nc.is_finalized():
        nc.finalize()
    return nc


_PROGRAM = None


def _get_program():
    global _PROGRAM
    if _PROGRAM is None:
        _PROGRAM = build_program()
    return _PROGRAM


def _patch_rows(x):
    """(C, R, Cc) padded map -> ((R-2)*(Cc-2), C*9) patch rows, (c,ki,kj)."""
    w = np.lib.stride_tricks.sliding_window_view(x, (PW, PW), axis=(1, 2))
    return np.ascontiguousarray(
        w.transpose(1, 2, 0, 3, 4).reshape((x.shape[1] - 2) * (x.shape[2] - 2), -1)
    )


def _host_prep(content_feats, style_feats):
    """Build per-core input maps."""
    f8 = mybir.dt.np(F8)
    cf = np.ascontiguousarray(np.asarray(content_feats, dtype=np.float32)[0])
    sf = np.ascontiguousarray(np.asarray(style_feats, dtype=np.float32)[0])
    cpad = np.pad(cf, ((0, 0), (1, 1), (1, 1)))
    spad = np.pad(sf, ((0, 0), (1, 1), (1, 1)))
    sprows = _patch_rows(spad)
    invn = 1.0 / np.maximum(
        np.linalg.norm(sprows, axis=1), np.float32(1e-12)
    ).astype(np.float32)
    snormq = (sprows * (np.float32(SCALE) * invn)[:, None]).astype(f8)
    snorm8 = np.ascontiguousarray(snormq.T).reshape(C, PW * PW, M)
    # quantize the content map once; rows are shifted views of the map
    cpadq = cpad.astype(f8).astype(np.float32)
    in_maps = []
    for i in range(NCORES):
        cslab = np.ascontiguousarray(cpadq[:, i * RPC : i * RPC + RPC + 2, :])
        crows = _patch_rows(cslab)                      # (NSH, 1152)
        cp8 = np.ascontiguousarray(crows.T).reshape(C, PW * PW, NSH)
        cp8 = np.concatenate(
            [cp8, np.zeros((C, 1, NSH), np.float32)], axis=1
        ).astype(f8)
        in_maps.append(
            {
                "snorm8": snorm8,
                "cp8": cp8,
                "sprows": sprows,
            }
        )
    return cf, in_maps


_DIVISOR = None


def _fold_divisor():
    global _DIVISOR
    if _DIVISOR is None:
        cnt = np.full(H, 3, dtype=np.float32)
        cnt[0] = cnt[-1] = 2
        _DIVISOR = np.outer(cnt, cnt).astype(np.float32) + np.float32(1e-8)
    return _DIVISOR


def _host_combine(cf, results):
    acc = np.zeros((C, H + 2, W), dtype=np.float32)
    for i in range(NCORES):
        acc[:, i * RPC : i * RPC + RPC + 2, :] += results[i]["racc_out"]
    recon = acc[:, 1 : 1 + H, :] / _fold_divisor()[None, :, :]
    diff = cf - recon
    return np.float32(np.mean(np.square(diff), dtype=np.float64))


def run(content_feats, style_feats, trace=False):
    nc = _get_program()
    cf, in_maps = _host_prep(content_feats, style_feats)
    res = run_bass_kernel_spmd(
        nc, in_maps, core_ids=list(range(NCORES)), trace=trace
    )
    mse = _host_combine(cf, res.results)
    return mse, res


def kernel(content_feats, style_feats):
    mse, _ = run(content_feats, style_feats)
    return np.array(mse, dtype=np.float32)


# revision 8
# speedup vs baseline: 1.9775x; 1.1190x over previous
"""CNN-MRF loss (retrieval kNN) on 8 Trainium2 NeuronCores.

Reference: cosine-similarity argmax between all 96x96 content patches and
96x96 style patches (3x3xC=128 patches, d=1152), gather matched style
patches, fold (overlap-add), MSE against content features.

Sharding: content-patch axis N split 8 ways (12 grid rows / core), style
replicated.  Per core:
  coarse: fp8(e4m3) DoubleRow matmuls on host-prenormalized style patch
     rows (scaled x1024, pair-interleaved planes) against fp8 content
     rows -- PSUM directly holds cosine scores.  Contraction 1152 = 5
     plane-pairs of 2x128 (10th plane zeros).  18 m-groups of 512 cols,
     PSUM double-bank tiles, drained to bf16 S rows by the Scalar engine.
     fp8 coarse top-1 picks a near-tied neighbor for ~6% of patches; MSE
     rel err ~6e-5 (validated on host).
  argmax: DVE tensor_tensor max tournament (2x bf16) -> per-group maxes,
     max8/max_index8 over 18 groups -> winning group; S rows round-trip
     through DRAM so an indirect DMA can fetch each partition's winning
     512-wide group; max_index8 over 512 -> local index.
  then: indirect-DMA gather of the matched (un-normalized fp32) style
     patch rows, PE transposes to channel-major, DVE fold accumulation
     into a 14-row output strip.  Tail work for tile j is emitted after
     tile j+1's matmuls so the PE never stalls on the argmax chain.
Host: sums the 8 overlapping strips, divides by fold counts, MSE.
"""
import sys
import numpy as np

for _p in ("/opt/trn_rl_repo",):
    if _p not in sys.path:
        sys.path.insert(0, _p)

import concourse.bass as bass
import concourse.bacc as bacc
import concourse.mybir as mybir
from concourse.bass import IndirectOffsetOnAxis
from concourse.bass_utils import run_bass_kernel_spmd
from concourse.tile import TileContext
from concourse.masks import make_identity

F32 = mybir.dt.float32
BF16 = mybir.dt.bfloat16
F8 = mybir.dt.float8e4
U32 = mybir.dt.uint32

C = 128          # channels
H = W = 96       # feature-map spatial dims
PW = 3           # patch size
HP = H + 2       # padded spatial
N = H * W        # content patches total (9216)
M = N            # style patches (9216)
D = C * PW * PW  # patch vector length (1152)
NCORES = 8
RPC = H // NCORES       # content grid rows per core (12)
NSH = RPC * W           # content patches per core (1152)
NT = NSH // 128         # n-tiles of 128 per core (9)
GW = 512                # m-group width (one PSUM bank of fp32)
NG = M // GW            # m-groups (18)
SG = 3                  # supergroups of 6 groups (3 double-bank tiles)
SCALE = 1024.0          # fp8 quantization scale for normalized style rows


def ts(i, size):
    return slice(i * size, (i + 1) * size)


def build_program():
    nc = bacc.Bacc()

    snorm8 = nc.declare_dram_parameter("snorm8", [C, 5, M, 2], F8, isOutput=False)
    cp8 = nc.declare_dram_parameter("cp8", [C, 10, NSH], F8, isOutput=False)
    sprows = nc.declare_dram_parameter("sprows", [M, D], F32, isOutput=False)
    prow18 = nc.declare_dram_parameter("prow18", [128, 1], U32, isOutput=False)
    idx_out = nc.declare_dram_parameter("idx_out", [NT, 128, 1], U32, isOutput=True)
    racc_out = nc.declare_dram_parameter(
        "racc_out", [C, RPC + 2, W], F32, isOutput=True
    )
    s_dram = nc.dram_tensor("s_scratch", [NT * 128 * NG, GW], BF16)

    DR = mybir.MatmulPerfMode.DoubleRow
    Copy = mybir.ActivationFunctionType.Copy
    MAX = mybir.AluOpType.max

    with TileContext(nc) as tc:
        with (
            tc.tile_pool(name="const", bufs=1) as constp,
            tc.tile_pool(name="big", bufs=1) as bigp,
            tc.tile_pool(name="work", bufs=2) as workp,
            tc.tile_pool(name="psD", bufs=3, space="PSUM") as psD,
            tc.tile_pool(name="psT", bufs=2, space="PSUM") as psT,
        ):
            ident = constp.tile([128, 128], F32)
            make_identity(nc, ident[:])
            prow_t = constp.tile([128, 1], U32)
            nc.sync.dma_start(out=prow_t[:], in_=prow18[:])

            # style rows: [c, pair, m, elem], plane 9 zeroed by the host
            snorm_t = bigp.tile([C, 5, M, 2], F8)
            for g in range(NG):
                nc.sync.dma_start(
                    out=snorm_t[:, :, ts(g, GW), :], in_=snorm8[:, :, ts(g, GW), :]
                )
            cp_t = bigp.tile([C, 10, NSH], F8)
            nc.sync.dma_start(out=cp_t[:], in_=cp8[:])

            racc = bigp.tile([C, RPC + 2, HP], F32)
            nc.gpsimd.memset(racc[:], 0.0)

            def emit_mm(j):
                """Coarse fp8 matmuls + scalar drains + S->DRAM dump."""
                S_sb = bigp.tile(
                    [C, NG * GW], BF16, tag="S_sb", bufs=2, name=f"S_{j}"
                )
                for sg in range(SG):
                    pds = [
                        psD.tile([128, 2 * GW], F32, tag="psD", name=f"pd_{j}_{sg}_{t}")
                        for t in range(3)
                    ]
                    for kp in range(5):
                        lhsT = cp_t[:, 2 * kp : 2 * kp + 2, ts(j, 128)]
                        for t in range(3):
                            for h in range(2):
                                g = sg * 6 + t * 2 + h
                                nc.tensor.matmul(
                                    out=pds[t][:, ts(h, GW)],
                                    lhsT=lhsT,
                                    rhs=snorm_t[:, kp, ts(g, GW), :].rearrange(
                                        "p n e -> p e n"
                                    ),
                                    start=(kp == 0),
                                    stop=(kp == 4),
                                    perf_mode=DR,
                                )
                    for t in range(3):
                        g0 = sg * 6 + t * 2
                        nc.scalar.activation(
                            S_sb[:, g0 * GW : (g0 + 2) * GW], pds[t][:], Copy
                        )
                    # dump this supergroup's rows for the later indirect fetch
                    nc.sync.dma_start(
                        out=s_dram[:]
                        .rearrange("(a p g) w -> a p g w", a=NT, p=128)[j][
                            :, ts(sg, 6), :
                        ]
                        .rearrange("p g w -> p (g w)"),
                        in_=S_sb[:, sg * 6 * GW : (sg + 1) * 6 * GW],
                    )
                return S_sb

            def emit_tail(j, S_sb):
                """Argmax, gathers, transpose + fold for tile j."""
                S3 = S_sb[:].rearrange("p (a b) -> p a b", b=GW)
                tmax = workp.tile([128, NG, 256], BF16, tag="tmax", name=f"tm_{j}")
                nc.vector.tensor_tensor(
                    out=tmax[:], in0=S3[:, :, 0:256], in1=S3[:, :, 256:512], op=MAX
                )
                w = 128
                while w >= 8:
                    nc.vector.tensor_tensor(
                        out=tmax[:, :, 0:w],
                        in0=tmax[:, :, 0:w],
                        in1=tmax[:, :, w : 2 * w],
                        op=MAX,
                    )
                    w //= 2
                gmax = workp.tile([128, NG], BF16, tag="gmax", name=f"gm_{j}")
                nc.vector.tensor_reduce(
                    gmax[:], tmax[:, :, 0:8], axis=mybir.AxisListType.X, op=MAX
                )
                gmax8 = workp.tile([128, 8], BF16, tag="gmax8", name=f"gm8_{j}")
                nc.vector.max(gmax8[:], gmax[:])
                gstar8 = workp.tile([128, 8], U32, tag="gstar8", name=f"gs8_{j}")
                nc.vector.max_index(gstar8[:], gmax8[:], gmax[:])

                rowid = workp.tile([128, 1], U32, tag="rowid", name=f"ri_{j}")
                nc.vector.tensor_tensor(
                    out=rowid[:], in0=prow_t[:], in1=gstar8[:, 0:1],
                    op=mybir.AluOpType.add,
                )
                rowid2 = workp.tile([128, 1], U32, tag="rowid2", name=f"ri2_{j}")
                nc.vector.tensor_scalar(
                    out=rowid2[:], in0=rowid[:], scalar1=j * 128 * NG, scalar2=None,
                    op0=mybir.AluOpType.add,
                )
                wrow = workp.tile([128, GW], BF16, tag="wrow", name=f"wr_{j}")
                nc.gpsimd.indirect_dma_start(
                    out=wrow[:],
                    out_offset=None,
                    in_=s_dram[:],
                    in_offset=IndirectOffsetOnAxis(ap=rowid2[:, 0:1], axis=0),
                )
                li8 = workp.tile([128, 8], U32, tag="li8", name=f"li_{j}")
                nc.vector.max_index(li8[:], gmax8[:], wrow[:])

                g512 = workp.tile([128, 1], U32, tag="g512", name=f"g5_{j}")
                nc.vector.tensor_scalar(
                    out=g512[:], in0=gstar8[:, 0:1], scalar1=GW, scalar2=None,
                    op0=mybir.AluOpType.mult,
                )
                bestu = workp.tile([128, 1], U32, tag="bestu", name=f"bu_{j}")
                nc.vector.tensor_tensor(
                    out=bestu[:], in0=g512[:], in1=li8[:, 0:1],
                    op=mybir.AluOpType.add,
                )
                nc.sync.dma_start(out=idx_out[j], in_=bestu[:])

                # gather matched (un-normalized) style patch rows (n-major)
                matched = workp.tile([128, D], F32, tag="matched", name=f"ma_{j}")
                nc.gpsimd.indirect_dma_start(
                    out=matched[:],
                    out_offset=None,
                    in_=sprows[:],
                    in_offset=IndirectOffsetOnAxis(ap=bestu[:, 0:1], axis=0),
                )
                matched3 = matched[:].rearrange("p (a b) -> p a b", b=9)

                # transpose to channel-major and fold-accumulate
                n0 = j * 128
                r0, c0 = n0 // W, n0 % W
                seg1 = (r0, c0, W - c0, 0)
                seg2 = (r0 + 1, 0, 128 - (W - c0), W - c0)
                for k in range(9):
                    ki, kj = k // 3, k % 3
                    psum_T = psT.tile([128, 128], F32, tag="psT", name=f"pT_{j}_{k}")
                    nc.tensor.transpose(psum_T[:], matched3[:, :, k], ident[:])
                    for (r, c, ln, off) in (seg1, seg2):
                        nc.vector.tensor_add(
                            racc[:, r + ki, c + kj : c + kj + ln],
                            racc[:, r + ki, c + kj : c + kj + ln],
                            psum_T[:, off : off + ln],
                        )

            # software pipeline: tail(j-1) is emitted after matmuls(j), so the
            # PE queue never waits on tile j-1's argmax/gather chain
            prev = None
            for j in range(NT):
                cur = emit_mm(j)
                if prev is not None:
                    emit_tail(j - 1, prev)
                prev = cur
            emit_tail(NT - 1, prev)

            nc.sync.dma_start(out=racc_out[:], in_=racc[:, :, 1 : 1 + W])

    if not nc.is_finalized():
        nc.finalize()
    return nc


_PROGRAM = None


def _get_program():
    global _PROGRAM
    if _PROGRAM is None:
        _PROGRAM = build_program()
    return _PROGRAM


def _patch_rows(x):
    """(C, R, Cc) padded map -> ((R-2)*(Cc-2), C*9) patch rows, (c,ki,kj)."""
    w = np.lib.stride_tricks.sliding_window_view(x, (PW, PW), axis=(1, 2))
    return np.ascontiguousarray(
        w.transpose(1, 2, 0, 3, 4).reshape((x.shape[1] - 2) * (x.shape[2] - 2), -1)
    )


def _pair_interleave(rows_T, n):
    """(1152, n) f32 plane-major -> (128, 5, n, 2) pair-interleaved."""
    a = rows_T.reshape(C, PW * PW, n)
    a = np.concatenate([a, np.zeros((C, 1, n), np.float32)], axis=1)
    return np.ascontiguousarray(a.reshape(C, 5, 2, n).transpose(0, 1, 3, 2))


def _host_prep(content_feats, style_feats):
    """Build per-core input maps."""
    f8 = mybir.dt.np(F8)
    cf = np.ascontiguousarray(np.asarray(content_feats, dtype=np.float32)[0])
    sf = np.ascontiguousarray(np.asarray(style_feats, dtype=np.float32)[0])
    cpad = np.pad(cf, ((0, 0), (1, 1), (1, 1)))
    spad = np.pad(sf, ((0, 0), (1, 1), (1, 1)))
    sprows = _patch_rows(spad)
    invn = 1.0 / np.maximum(
        np.linalg.norm(sprows, axis=1), np.float32(1e-12)
    ).astype(np.float32)
    snormq = (
        (sprows * (np.float32(SCALE) * invn)[:, None]).astype(f8).astype(np.float32)
    )
    snorm8 = _pair_interleave(np.ascontiguousarray(snormq.T), M).astype(f8)
    # quantize the content map once; rows are shifted views of the map
    cpadq = cpad.astype(f8).astype(np.float32)
    prow = (np.arange(128, dtype=np.uint32) * NG).reshape(128, 1)
    in_maps = []
    for i in range(NCORES):
        cslab = np.ascontiguousarray(cpadq[:, i * RPC : i * RPC + RPC + 2, :])
        crows = _patch_rows(cslab)                      # (NSH, 1152)
        cp8 = np.ascontiguousarray(crows.T).reshape(C, PW * PW, NSH)
        cp8 = np.concatenate(
            [cp8, np.zeros((C, 1, NSH), np.float32)], axis=1
        ).astype(f8)
        in_maps.append(
            {
                "snorm8": snorm8,
                "cp8": cp8,
                "sprows": sprows,
                "prow18": prow,
            }
        )
    return cf, in_maps


_DIVISOR = None


def _fold_divisor():
    global _DIVISOR
    if _DIVISOR is None:
        cnt = np.full(H, 3, dtype=np.float32)
        cnt[0] = cnt[-1] = 2
        _DIVISOR = np.outer(cnt, cnt).astype(np.float32) + np.float32(1e-8)
    return _DIVISOR


def _host_combine(cf, results):
    acc = np.zeros((C, H + 2, W), dtype=np.float32)
    for i in range(NCORES):
        acc[:, i * RPC : i * RPC + RPC + 2, :] += results[i]["racc_out"]
    recon = acc[:, 1 : 1 + H, :] / _fold_divisor()[None, :, :]
    diff = cf - recon
    return np.float32(np.mean(np.square(diff), dtype=np.float64))


def run(content_feats, style_feats, trace=False):
    nc = _get_program()
    cf, in_maps = _host_prep(content_feats, style_feats)
    res = run_bass_kernel_spmd(
        nc, in_maps, core_ids=list(range(NCORES)), trace=trace
    )
    mse = _host_combine(cf, res.results)
    return mse, res


def kernel(content_feats, style_feats):
    mse, _ = run(content_feats, style_feats)
    return np.array(mse, dtype=np.float32)
